# revision 7
# baseline (speedup 1.0000x reference)
"""Bidirectional Mamba2 layer on 8 NeuronCores.

Sharding: 8 cores = 4 batch elements x 2 directions (fw/bw). Each core runs
one full Mamba2 layer pass on one sequence; the host flips the bw sequences,
adds fw+bw results, and applies the padding mask.

Per-core kernel (L=2048, chunked SSD scan with T=128). Structure tuned for
TRN2 per-instruction overheads (~600ns DVE bubble, ~1us Pool floor, ~2.7us
ACT table-set switches):
  1. dt block of in_proj, softplus via the native ACT table.
  2. per xBC channel tile t: in_proj matmuls (all 4 tb) -> full-L conv taps
     (DVE) -> Silu (ACT, one table run) -> DRAM roundtrip write. in_proj of
     tile t+1 overlaps conv of tile t.
  3. chunked scan, ops batched over all 16 heads per chunk: utmp16/m16 as
     single [128,16,128] DVE ops (segsum mask folded into gt), xdt=x*dt ->
     xdw=xdt*decay chain, state matmul batched 8 heads/matmul, D*x folded
     in-place into conv output. z in_proj matmuls interleaved 2 blocks per
     chunk with raw-copy eviction (no ACT table thrash).
  4. late phase: one grouped Silu pass over z, gating in place, RMSNorm via
     Ln/Exp (one table set), out_proj per tb (norm_w folded into w_out).
"""

import numpy as np

D_MODEL = 512
D_STATE = 128
NH = 16
HD = 64
D_INNER = 1024
D_XBC = 1280
D_IN = 2320
L = 2048
T = 128
NCH = L // T
B_SZ = 4
EPS = 1e-5

_CACHE = {}


def _patch_drain(tile, mybir, ScopedClock):
    # workaround: this walrus build rejects >2 sem waits per instruction;
    # spread the TileContext exit-drain waits across nop instructions.
    def _drain_and_barrier(self, tick_clock, wait_clock):
        nc_ = self.nc
        probe = nc_.sync.nop()
        wait_clock.add_sem_waits(
            probe.ins, ScopedClock({None: tick_clock.global_clock})
        )
        waits = list(probe.ins.sync_info.on_wait or [])
        if probe.ins.sync_info is not None:
            probe.ins.sync_info.on_wait = waits[:1]
            rest = waits[1:]
        else:
            rest = []
        for w in rest:
            n = nc_.sync.nop()
            if n.ins.sync_info is None:
                n.ins.sync_info = mybir.SyncInfo(on_wait=[w], on_update=[])
            else:
                n.ins.sync_info.on_wait = [w]
        nc_.sync.drain()
        nc_.all_engine_barrier()
        assert self.sems is not None
        popped = nc_._tile_sem_poison_stack.pop()
        assert popped is self._sem_poison
        nc_.clear_and_free_semaphores(list(self.sems.allocated().values()))
        nc_.all_engine_barrier()

    tile.TileContext._drain_and_barrier = _drain_and_barrier


def _build_program():
    import concourse.bass as bass
    import concourse.mybir as mybir
    import concourse.tile as tile
    from concourse.vector_clock import ScopedClock

    _patch_drain(tile, mybir, ScopedClock)

    f32 = mybir.dt.float32
    bf16 = mybir.dt.bfloat16
    AF = mybir.ActivationFunctionType
    OP = mybir.AluOpType

    nc = bass.Bass("TRN2", target_bir_lowering=False, debug=False)

    # ---------------- DRAM I/O ----------------
    xT_d = nc.dram_tensor("xT", [D_MODEL, L], bf16, kind="ExternalInput")
    w_in_d = nc.dram_tensor("w_in", [D_MODEL, D_IN], bf16, kind="ExternalInput")
    w_out_d = nc.dram_tensor("w_out", [D_INNER, D_MODEL], bf16, kind="ExternalInput")
    convw_d = nc.dram_tensor("convw", [128, 10, 4], f32, kind="ExternalInput")
    convb_d = nc.dram_tensor("convb", [128, 10], f32, kind="ExternalInput")
    dtb_d = nc.dram_tensor("dtb", [16, 1], f32, kind="ExternalInput")
    nae_d = nc.dram_tensor("nae", [16, 1], f32, kind="ExternalInput")  # -exp(A_log)
    dcol_d = nc.dram_tensor("dcol", [128, 8], f32, kind="ExternalInput")  # D per pair-tile
    alow_d = nc.dram_tensor("alow", [128, 128], bf16, kind="ExternalInput")
    uinc_d = nc.dram_tensor("uinc", [128, 128], bf16, kind="ExternalInput")
    idnb_d = nc.dram_tensor("idnb", [128, 128], bf16, kind="ExternalInput")
    idnf_d = nc.dram_tensor("idnf", [128, 128], f32, kind="ExternalInput")
    ones_d = nc.dram_tensor("ones", [128, 1], bf16, kind="ExternalInput")
    onesrf_d = nc.dram_tensor("onesrf", [1, 128], f32, kind="ExternalInput")
    onesrb_d = nc.dram_tensor("onesrb", [1, 128], bf16, kind="ExternalInput")
    yT_d = nc.dram_tensor("yT", [D_MODEL, L], f32, kind="ExternalOutput")

    with tile.TileContext(nc) as tc:
        with (
            tc.tile_pool(name="const", bufs=1) as cpool,
            tc.tile_pool(name="dram", bufs=1, space="DRAM") as dpool,
            tc.tile_pool(name="mid", bufs=1) as mid,
            tc.tile_pool(name="pA", bufs=1) as pA,
        ):
            # ---------------- constants ----------------
            ALOW = cpool.tile([128, 128], bf16, tag="alow")
            nc.sync.dma_start(ALOW[:], alow_d.ap())
            UINC = cpool.tile([128, 128], bf16, tag="uinc")
            nc.sync.dma_start(UINC[:], uinc_d.ap())
            IDNB = cpool.tile([128, 128], bf16, tag="idnb")
            nc.sync.dma_start(IDNB[:], idnb_d.ap())
            IDNF = cpool.tile([128, 128], f32, tag="idnf")
            nc.sync.dma_start(IDNF[:], idnf_d.ap())
            ONEC = cpool.tile([128, 1], bf16, tag="ones")
            nc.sync.dma_start(ONEC[:], ones_d.ap())
            ONESRF = cpool.tile([1, 128], f32, tag="onesrf")
            nc.sync.dma_start(ONESRF[:], onesrf_d.ap())
            ONESRB = cpool.tile([1, 128], bf16, tag="onesrb")
            nc.sync.dma_start(ONESRB[:], onesrb_d.ap())
            CONVW = cpool.tile([128, 10, 4], f32, tag="convw")
            nc.sync.dma_start(CONVW[:], convw_d.ap())
            CONVB = cpool.tile([128, 10], f32, tag="convb")
            nc.sync.dma_start(CONVB[:], convb_d.ap())
            DTB = cpool.tile([16, 1], f32, tag="dtb")
            nc.sync.dma_start(DTB[:], dtb_d.ap())
            NAE = cpool.tile([16, 1], f32, tag="nae")
            nc.sync.dma_start(NAE[:], nae_d.ap())
            DCOL = cpool.tile([128, 8], f32, tag="dcol")
            nc.sync.dma_start(DCOL[:], dcol_d.ap())
            EPSC = cpool.tile([128, 1], f32, tag="epsc")
            nc.vector.memset(EPSC[:], EPS)

            # ---------------- persistent tensors ----------------
            dtld = mid.tile([80, L], f32, tag="dtld")           # dt 0:16, raw 32:48, logdA 64:80
            dtldT = mid.tile([128, NCH, 80], f32, tag="dtldT")  # time-major dt/logdA
            atot = mid.tile([16, 16], f32, tag="atot")          # [head, chunk]
            atotT = mid.tile([16, 16], f32, tag="atotT")        # [chunk, head]
            s_sb = [mid.tile([128, NH, HD], bf16, tag=f"s_sb{i}", name=f"s_sb{i}")
                    for i in range(2)]
            atotF = mid.tile([1, 256], f32, tag="atotF")
            wdin_all = mid.tile([128, NCH, 32], f32, tag="wdin_all")
            atb_all = mid.tile([128, NCH, 16], f32, tag="atb_all")
            sz = mid.tile([128, 8, L], bf16, tag="sz")          # raw z, then silu/g/gn

            rt_dram = dpool.tile([D_XBC, L], bf16)              # roundtrip buffer

            xTr = xT_d.ap().rearrange("(ko p) t -> p ko t", p=128)
            wir = w_in_d.ap().rearrange("(ko p) m -> p ko m", p=128)
            xTs = pA.tile([128, 4, L], bf16, tag="xTs")
            wis = pA.tile([128, 4, D_IN], bf16, tag="wis")
            for k in range(4):
                nc.sync.dma_start(xTs[:, k, :], xTr[:, k, :])
                nc.sync.dma_start(wis[:, k, :], wir[:, k, :])

            with tc.tile_pool(name="p_ysb", bufs=1) as p_ysb:
                y_sb = p_ysb.tile([128, 8, L], bf16, tag="y_sb")
                with tc.tile_pool(name="p_xbc", bufs=1) as p_xbc:
                    xbc_x = p_xbc.tile([128, 8, L], bf16, tag="xbc_x")
                    xbc_B = p_xbc.tile([128, L], bf16, tag="xbc_B")
                    xbc_C = p_xbc.tile([128, L], bf16, tag="xbc_C")

                    with (
                        tc.tile_pool(name="p_pre", bufs=2) as p_pre,
                        tc.tile_pool(name="pC", bufs=1) as pC,
                        tc.tile_pool(name="pS", bufs=2) as pS,
                        tc.tile_pool(name="pS1", bufs=1) as pS1,
                        tc.tile_pool(name="pXB", bufs=2) as pXB,
                        tc.tile_pool(name="psIn", bufs=2, space="PSUM") as psIn,
                        tc.tile_pool(name="psY", bufs=1, space="PSUM") as psY,
                        tc.tile_pool(name="psS", bufs=1, space="PSUM") as psS,
                        tc.tile_pool(name="psE", bufs=2, space="PSUM") as psE,
                    ):
                        # ---- dt block of in_proj (m = 18), all tb ----
                        for tb in range(4):
                            tsl = slice(tb * 512, (tb + 1) * 512)
                            ps = psIn.tile([128, 512], f32, tag="ps_in")
                            for k in range(4):
                                nc.tensor.matmul(
                                    ps[:16, :], wis[:, k, 18 * 128: 18 * 128 + 16],
                                    xTs[:, k, tsl], start=(k == 0), stop=(k == 3))
                            nc.scalar.copy(dtld[32:48, tsl], ps[:16, :])
                        # dt = softplus(pre) = ln(1 + exp(pre + dtb))
                        nc.scalar.activation(dtld[32:48, :], dtld[32:48, :], AF.Exp,
                                             bias=DTB[:, 0:1])
                        nc.scalar.activation(dtld[0:16, :], dtld[32:48, :], AF.Ln,
                                             bias=1.0)
                        # logdA = -exp(A_log) * dt   (f32)
                        nc.vector.tensor_scalar_mul(
                            dtld[64:80, :], dtld[0:16, :], NAE[:, 0:1])

                        # Atot per chunk = exp(chunk-sums of logdA)
                        red = psIn.tile([128, 512], f32, tag="ps_in", name="red")
                        nc.vector.tensor_reduce(
                            red[0:16, 0:16],
                            dtld[64:80, :].rearrange("p (c t) -> p c t", c=NCH),
                            op=OP.add, axis=mybir.AxisListType.X,
                        )
                        nc.scalar.activation(atot[:], red[0:16, 0:16], AF.Exp)
                        atT_ps = psIn.tile([128, 512], f32, tag="ps_in", name="atT_ps")
                        nc.tensor.transpose(
                            atT_ps[0:16, 0:16], atot[:], IDNF[0:16, 0:16])
                        nc.vector.tensor_copy(atotT[:], atT_ps[0:16, 0:16])
                        nc.sync.dma_start(
                            atotF[:].rearrange("p (c h) -> p c h", c=16), atotT[:])

                        # time-major dt/logdA per chunk via PE transpose
                        for c in range(NCH):
                            trp = psIn.tile([128, 512], f32, tag="ps_in", name="trp")
                            nc.tensor.transpose(
                                trp[:, 0:80], dtld[:, c * T:(c + 1) * T],
                                IDNF[0:80, 0:80])
                            nc.vector.tensor_copy(dtldT[:, c, :], trp[:, 0:80])

                        # ==== in_proj xBC (t-major) fused with full-L conv ====
                        for t in [8, 9] + list(range(8)):
                            m = 8 + t
                            pre = p_pre.tile([128, L + 3], bf16, tag="pre")
                            nc.vector.memset(pre[:, 0:3], 0.0)
                            for tb in range(4):
                                tsl = slice(tb * 512, (tb + 1) * 512)
                                ps = psIn.tile([128, 512], f32, tag="ps_in")
                                for k in range(4):
                                    nc.tensor.matmul(
                                        ps[:], wis[:, k, m * 128:(m + 1) * 128],
                                        xTs[:, k, tsl],
                                        start=(k == 0), stop=(k == 3))
                                nc.scalar.copy(pre[:, 3 + tb * 512: 3 + (tb + 1) * 512],
                                               ps[:])
                            acc = pC.tile([128, L], bf16, tag="conv_acc")
                            nc.vector.tensor_scalar_mul(
                                acc[:], pre[:, 0:L], CONVW[:, t, 0:1])
                            for k in (1, 2, 3):
                                nc.vector.scalar_tensor_tensor(
                                    acc[:], pre[:, k:k + L],
                                    CONVW[:, t, k:k + 1], acc[:],
                                    op0=OP.mult, op1=OP.add)
                            if t < 8:
                                dest = xbc_x[:, t, :]
                            elif t == 8:
                                dest = xbc_B[:]
                            else:
                                dest = xbc_C[:]
                            nc.scalar.activation(dest, acc[:], AF.Silu,
                                                 bias=CONVB[:, t:t + 1])
                            nc.sync.dma_start(
                                rt_dram[t * 128:(t + 1) * 128, :], dest)

                        # ---- fold D into x in-place (y eviction adds it) ----
                        for half in range(2):
                            hsl = slice(half * 1024, (half + 1) * 1024)
                            nc.vector.tensor_tensor(
                                xbc_x[:, :, hsl], xbc_x[:, :, hsl],
                                DCOL[:, :, None].to_broadcast([128, 8, 1024]),
                                op=OP.mult)

                        # ---- per-chunk decay prep (grouped Exp) ----
                        for c in range(NCH):
                            ld_bf = pS.tile([128, 16], bf16, tag="ld_bf")
                            nc.vector.tensor_copy(ld_bf[:], dtldT[:, c, 64:80])
                            wd_ps = psE.tile([128, 4, 128], f32, tag="ps_e",
                                             name="wd_ps")
                            nc.tensor.matmul(wd_ps[:, 0, 0:16], ALOW[:], ld_bf[:],
                                             start=True, stop=True)
                            nc.tensor.matmul(wd_ps[:, 0, 16:32], UINC[:], ld_bf[:],
                                             start=True, stop=True)
                            nc.scalar.activation(wdin_all[:, c, :],
                                                 wd_ps[:, 0, 0:32], AF.Exp)
                            if c > 0:
                                nc.tensor.matmul(
                                    wd_ps[:, 0, 32:48], ONESRF[:],
                                    atotF[0:1, c * 16:(c + 1) * 16],
                                    start=True, stop=True)
                                nc.vector.tensor_copy(atb_all[:, c, :],
                                                      wd_ps[:, 0, 32:48])

                        # ======================= scan =======================
                        for c in range(NCH):
                            csl = slice(c * T, (c + 1) * T)
                            wdin = wdin_all[:, c, :]

                            # interleave 2 z-blocks of in_proj per chunk
                            for zi in (2 * c, 2 * c + 1):
                                zm, ztb = zi % 8, zi // 8
                                ztsl = slice(ztb * 512, (ztb + 1) * 512)
                                zps = psIn.tile([128, 512], f32, tag="ps_in")
                                for k in range(4):
                                    nc.tensor.matmul(
                                        zps[:], wis[:, k, zm * 128:(zm + 1) * 128],
                                        xTs[:, k, ztsl],
                                        start=(k == 0), stop=(k == 3))
                                nc.scalar.copy(sz[:, zm, ztsl], zps[:])

                            xbt = pXB.tile([128, D_XBC], bf16, tag="xbt")
                            nc.sync.dma_start_transpose(xbt[:, 1024:1152],
                                                        rt_dram[1024:1152, csl])
                            nc.sync.dma_start_transpose(xbt[:, 1152:1280],
                                                        rt_dram[1152:1280, csl])
                            nc.sync.dma_start_transpose(xbt[:, 0:1024],
                                                        rt_dram[0:1024, csl])

                            # utmp16[k,h,i] = logdA[k,h] * [k<=i]  (one DVE op)
                            utmp16 = pS1.tile([128, NH, 128], bf16, tag="utmp16")
                            nc.vector.tensor_tensor(
                                utmp16[:],
                                UINC[:, None, :].to_broadcast([128, NH, 128]),
                                dtldT[:, c, 64:80][:, :, None]
                                .to_broadcast([128, NH, 128]),
                                op=OP.mult)

                            # Gt = B @ C^T, tril-masked (shared across heads)
                            gt_ps = psE.tile([128, 4, 128], f32, tag="ps_e",
                                             name="gt_ps")
                            nc.tensor.matmul(gt_ps[:, 0, :], xbc_B[:, csl],
                                             xbc_C[:, csl], start=True, stop=True)
                            gt = pS.tile([128, 128], bf16, tag="gt")
                            nc.vector.tensor_tensor(gt[:], gt_ps[:, 0, :],
                                                    UINC[:], op=OP.mult)

                            # segsum -> exp into m16, then m16 *= gt (broadcast)
                            m16 = pS.tile([128, NH, 128], bf16, tag="m16")
                            for hb in range(4):
                                e_ps = psE.tile([128, 4, 128], f32, tag="ps_e",
                                                name="e_ps")
                                nc.tensor.matmul(e_ps[:], ALOW[:],
                                                 utmp16[:, 4 * hb:4 * hb + 4, :],
                                                 start=True, stop=True)
                                nc.scalar.activation(m16[:, 4 * hb:4 * hb + 4, :],
                                                     e_ps[:], AF.Exp)
                            nc.vector.tensor_tensor(
                                m16[:], gt[:, None, :].to_broadcast([128, NH, 128]),
                                m16[:], op=OP.mult)

                            # xdt = x * dt, xdw = xdt * decay (16 heads at once)
                            xdt = pS1.tile([128, NH, HD], bf16, tag="xdt")
                            nc.vector.tensor_tensor(
                                xdt[:],
                                xbt[:, 0:1024].rearrange("p (h d) -> p h d", h=16),
                                dtldT[:, c, 0:16][:, :, None]
                                .to_broadcast([128, NH, HD]),
                                op=OP.mult)
                            xdw = pS1.tile([128, NH, HD], bf16, tag="xdw")
                            nc.gpsimd.tensor_tensor(
                                xdw[:], xdt[:],
                                wdin[:, 0:16][:, :, None]
                                .to_broadcast([128, NH, HD]),
                                op=OP.mult)

                            y_ps = psY.tile([128, 8, T], f32, tag="y_ps")
                            s_ps = psS.tile([128, NH, HD], f32, tag="s_ps")

                            for hb in range(4):
                                if c > 0:
                                    ddiag4 = pS.tile([128, 4, 128], bf16,
                                                     tag="ddiag4")
                                    nc.gpsimd.tensor_tensor(
                                        ddiag4[:],
                                        IDNB[:, None, :]
                                        .to_broadcast([128, 4, 128]),
                                        wdin[:, 16 + 4 * hb: 20 + 4 * hb]
                                        [:, :, None].to_broadcast([128, 4, 128]),
                                        op=OP.mult)
                                    cd_ps = psE.tile([128, 4, 128], f32,
                                                     tag="ps_e", name="cd_ps")
                                    nc.tensor.matmul(cd_ps[:], xbt[:, 1152:1280],
                                                     ddiag4[:], start=True,
                                                     stop=True)
                                    cd_sb = pS.tile([128, 4, 128], bf16,
                                                    tag="cd_sb")
                                    nc.scalar.copy(cd_sb[:], cd_ps[:])

                                for hq in range(4):
                                    h = hb * 4 + hq
                                    ph, fh = (h % 2) * 64, h // 2
                                    nc.tensor.matmul(
                                        y_ps[ph:ph + 64, fh, :],
                                        xdt[:, h, :], m16[:, h, :],
                                        start=True, stop=(c == 0))
                                    if c > 0:
                                        nc.tensor.matmul(
                                            y_ps[ph:ph + 64, fh, :],
                                            s_sb[(c + 1) % 2][:, h, :],
                                            cd_sb[:, hq, :],
                                            start=False, stop=True,
                                            skip_group_check=True)

                            # state: S_chunk = B^T @ (x*dt*w), 8 heads per matmul
                            nc.tensor.matmul(s_ps[:, 0:8, :], xbt[:, 1024:1152],
                                             xdw[:, 0:8, :], start=True, stop=True)
                            nc.tensor.matmul(s_ps[:, 8:16, :], xbt[:, 1024:1152],
                                             xdw[:, 8:16, :], start=True,
                                             stop=True)

                            # y eviction: y = D*x + y_psum (D pre-folded into x)
                            nc.vector.tensor_tensor(
                                y_sb[:, :, csl], xbc_x[:, :, csl], y_ps[:],
                                op=OP.add)

                            # state evac: S_new = S_old * atot + S_psum
                            if c == 0:
                                nc.vector.tensor_copy(s_sb[0][:], s_ps[:])
                            else:
                                s_scaled = pS.tile([128, NH, HD], bf16,
                                                   tag="s_scaled")
                                nc.gpsimd.tensor_tensor(
                                    s_scaled[:], s_sb[(c + 1) % 2][:],
                                    atb_all[:, c, :][:, :, None]
                                    .to_broadcast([128, NH, HD]),
                                    op=OP.mult)
                                nc.vector.tensor_tensor(
                                    s_sb[c % 2][:], s_scaled[:], s_ps[:],
                                    op=OP.add)

                # ==== late phase: silu(z), gating, RMSNorm, out_proj ====
                with (
                    tc.tile_pool(name="p_late", bufs=1) as p_late,
                    tc.tile_pool(name="pG", bufs=2) as pG,
                    tc.tile_pool(name="pO", bufs=2) as pO,
                    tc.tile_pool(name="psO", bufs=2, space="PSUM") as psO,
                    tc.tile_pool(name="psN", bufs=2, space="PSUM") as psN,
                ):
                    wo = p_late.tile([128, 8, D_MODEL], bf16, tag="wo")
                    rstd_cols = p_late.tile([128, 16], f32, tag="rstd_cols")
                    rstdT = p_late.tile([16, 128], bf16, tag="rstdT")
                    rstdF = p_late.tile([1, L], bf16, tag="rstdF")
                    wor = w_out_d.ap().rearrange("(ko p) m -> p ko m", p=128)
                    nc.sync.dma_start(wo[:], wor)

                    # grouped Silu over all raw z (one table run)
                    for half in range(2):
                        hsl = slice(half * 1024, (half + 1) * 1024)
                        nc.scalar.activation(sz[:, :, hsl], sz[:, :, hsl], AF.Silu)

                    for tb in range(4):
                        tsl = slice(tb * 512, (tb + 1) * 512)
                        # gating: g = y * silu(z), in place into sz
                        nc.vector.tensor_tensor(sz[:, :, tsl], sz[:, :, tsl],
                                                y_sb[:, :, tsl], op=OP.mult)
                        # RMSNorm factors per chunk
                        for ci in range(4):
                            c = 4 * tb + ci
                            csl = slice(c * T, (c + 1) * T)
                            g2 = pG.tile([128, 8, T], bf16, tag="g2")
                            nc.scalar.activation(g2[:], sz[:, :, csl], AF.Square)
                            ssn = psN.tile([128, 128], f32, tag="ps_n")
                            for t in range(8):
                                nc.tensor.matmul(ssn[:, 0:1], g2[:, t, :], ONEC[:],
                                                 start=(t == 0), stop=(t == 7))
                            lnv = pG.tile([128, 1], f32, tag="lnv")
                            nc.scalar.activation(lnv[:], ssn[:, 0:1], AF.Ln,
                                                 bias=EPSC[:, 0:1],
                                                 scale=1.0 / D_INNER)
                            nc.scalar.activation(rstd_cols[:, c:c + 1], lnv[:],
                                                 AF.Exp, scale=-0.5)
                        # transpose + broadcast rstd over channels
                        rsn = psN.tile([128, 128], f32, tag="ps_n", name="rsn")
                        nc.tensor.transpose(rsn[0:4, 0:128],
                                            rstd_cols[:, 4 * tb:4 * tb + 4], IDNF[:])
                        nc.vector.tensor_copy(rstdT[0:4, :], rsn[0:4, 0:128])
                        nc.sync.dma_start(
                            rstdF[0:1, tsl].rearrange("p (c t) -> p c t", c=4),
                            rstdT[0:4, :])
                        rstd_bc = pG.tile([128, 512], bf16, tag="rstd_bc")
                        for ci in range(4):
                            c = 4 * tb + ci
                            rbn = psN.tile([128, 128], f32, tag="ps_n", name="rbn")
                            nc.tensor.matmul(rbn[:], ONESRB[:],
                                             rstdF[0:1, c * T:(c + 1) * T],
                                             start=True, stop=True)
                            nc.vector.tensor_copy(rstd_bc[:, ci * T:(ci + 1) * T],
                                                  rbn[:])
                        # gn = g * rstd (norm_w folded into w_out on host)
                        nc.vector.tensor_tensor(
                            sz[:, :, tsl], sz[:, :, tsl],
                            rstd_bc[:, None, :].to_broadcast([128, 8, 512]),
                            op=OP.mult)
                        # out_proj
                        for mo in range(4):
                            ps = psO.tile([128, 512], f32, tag="ps_o")
                            for k in range(8):
                                nc.tensor.matmul(
                                    ps[:], wo[:, k, mo * 128:(mo + 1) * 128],
                                    sz[:, k, tsl], start=(k == 0), stop=(k == 7))
                            yTs = pO.tile([128, 512], f32, tag="yTs")
                            nc.scalar.copy(yTs[:], ps[:])
                            nc.sync.dma_start(
                                yT_d.ap()[mo * 128:(mo + 1) * 128, tsl], yTs[:])

    _fix_waits(nc, mybir)

    return nc


def _fix_waits(nc, mybir):
    """This walrus build supports one sem-wait slot per instruction; hoist
    excess waits onto preceding NoOps on the same engine."""
    nwn = [0]
    for bb in nc.main_func.blocks:
        newl = []
        changed = False
        for inst in bb.instructions:
            si = inst.sync_info
            waits = list(si.on_wait) if (si and si.on_wait) else []
            if len(waits) > 1:
                imm = [w for w in waits if w.wait_reg is None]
                reg = [w for w in waits if w.wait_reg is not None]
                keep = (reg + imm)[:1]
                spill = [w for w in waits if w not in keep]
                assert not any(w.wait_reg is not None for w in spill), inst.name
                for w in spill:
                    nwn[0] += 1
                    nop = mybir.InstNoOp(name=f"I-wsplit-{nwn[0]}", ins=[], outs=[])
                    nop.engine = inst.engine
                    nop.sync_info = mybir.SyncInfo(on_wait=[w], on_update=[])
                    nc.register_instruction(nop)
                    newl.append(nop)
                si.on_wait = keep
                changed = True
            newl.append(inst)
        if changed:
            bb.instructions = newl
    return nc


def _get_program():
    if "nc" not in _CACHE:
        _CACHE["nc"] = _build_program()
    return _CACHE["nc"]


def _host_consts():
    if "consts" in _CACHE:
        return _CACHE["consts"]
    import ml_dtypes
    k = np.arange(128)
    alow = (k[:, None] > k[None, :]).astype(np.float32)      # [k > j]
    uinc = (k[:, None] <= k[None, :]).astype(np.float32)     # [k <= i]
    idn = np.eye(128, dtype=np.float32)
    consts = dict(
        alow=alow.astype(ml_dtypes.bfloat16),
        uinc=uinc.astype(ml_dtypes.bfloat16),
        idnb=idn.astype(ml_dtypes.bfloat16),
        idnf=idn,
        ones=np.ones((128, 1), ml_dtypes.bfloat16),
        onesrf=np.ones((1, 128), np.float32),
        onesrb=np.ones((1, 128), ml_dtypes.bfloat16),
    )
    _CACHE["consts"] = consts
    return consts


def _core_inputs(x_seq, p):
    """x_seq: (L, D_MODEL) f32 (already flipped for bw); p: dict of params."""
    import ml_dtypes
    consts = _host_consts()
    dcol = p["D"].astype(np.float32).repeat(HD).reshape(8, 128).T.copy()
    convw = np.ascontiguousarray(
        p["conv_w"].astype(np.float32).reshape(4, 10, 128).transpose(2, 1, 0)
    )
    convb = np.ascontiguousarray(p["conv_b"].astype(np.float32).reshape(10, 128).T)
    w_out = (p["norm_w"].astype(np.float32)[:, None]
             * p["out_proj"].astype(np.float32))
    return dict(
        xT=np.ascontiguousarray(x_seq.T).astype(ml_dtypes.bfloat16),
        w_in=np.ascontiguousarray(p["in_proj"]).astype(ml_dtypes.bfloat16),
        w_out=np.ascontiguousarray(w_out).astype(ml_dtypes.bfloat16),
        convw=convw,
        convb=convb,
        dtb=p["dt_bias"].astype(np.float32).reshape(16, 1),
        nae=(-np.exp(p["A_log"].astype(np.float32))).reshape(16, 1),
        dcol=dcol,
        **consts,
    )


def kernel(**inputs):
    from concourse.bass_utils import run_bass_kernel_spmd

    nc = _get_program()
    x = np.asarray(inputs["x"], np.float32)
    mask = np.asarray(inputs["padding_mask"])

    def params(pre):
        names = ["in_proj", "conv_w", "conv_b", "dt_bias", "A_log", "D", "norm_w", "out_proj"]
        return {n: np.asarray(inputs[pre + n]) for n in names}

    pf, pb = params("fw_"), params("bw_")
    in_maps = []
    for b in range(B_SZ):
        in_maps.append(_core_inputs(x[b], pf))
    for b in range(B_SZ):
        in_maps.append(_core_inputs(x[b][::-1], pb))

    res = run_bass_kernel_spmd(nc, in_maps, core_ids=list(range(8)))
    out = np.empty((B_SZ, L, D_MODEL), np.float32)
    for b in range(B_SZ):
        yf = res.results[b]["yT"].T
        yb = res.results[B_SZ + b]["yT"].T[::-1]
        out[b] = yf + yb
    out[mask] = 0.0
    return out


# revision 13
# speedup vs baseline: 1.0351x; 1.0351x over previous
"""Bidirectional Mamba2 layer on 8 NeuronCores.

Sharding: 8 cores = 4 batch elements x 2 directions (fw/bw). Each core runs
one full Mamba2 layer pass on one sequence; the host flips the bw sequences,
adds fw+bw results, and applies the padding mask.

Per-core kernel (L=2048, chunked SSD scan with T=128). Structure tuned for
TRN2 per-instruction overheads (~600ns DVE bubble, ~1us Pool floor, ~2.7us
ACT table-set switches):
  1. dt block of in_proj, softplus via the native ACT table.
  2. per xBC channel tile t: in_proj matmuls (all 4 tb) -> full-L conv taps
     (DVE) -> Silu (ACT, one table run) -> DRAM roundtrip write. in_proj of
     tile t+1 overlaps conv of tile t.
  3. chunked scan, ops batched over all 16 heads per chunk: utmp16/m16 as
     single [128,16,128] DVE ops (segsum mask folded into gt), xdt=x*dt ->
     xdw=xdt*decay chain, state matmul batched 8 heads/matmul, D*x folded
     in-place into conv output. z in_proj matmuls interleaved 2 blocks per
     chunk with raw-copy eviction (no ACT table thrash).
  4. late phase: one grouped Silu pass over z, gating in place, RMSNorm via
     Ln/Exp (one table set), out_proj per tb (norm_w folded into w_out).
"""

import numpy as np

D_MODEL = 512
D_STATE = 128
NH = 16
HD = 64
D_INNER = 1024
D_XBC = 1280
D_IN = 2320
L = 2048
T = 128
NCH = L // T
B_SZ = 4
EPS = 1e-5
INTER_OLD = True

_CACHE = {}


def _patch_drain(tile, mybir, ScopedClock):
    # workaround: this walrus build rejects >2 sem waits per instruction;
    # spread the TileContext exit-drain waits across nop instructions.
    def _drain_and_barrier(self, tick_clock, wait_clock):
        nc_ = self.nc
        probe = nc_.sync.nop()
        wait_clock.add_sem_waits(
            probe.ins, ScopedClock({None: tick_clock.global_clock})
        )
        waits = list(probe.ins.sync_info.on_wait or [])
        if probe.ins.sync_info is not None:
            probe.ins.sync_info.on_wait = waits[:1]
            rest = waits[1:]
        else:
            rest = []
        for w in rest:
            n = nc_.sync.nop()
            if n.ins.sync_info is None:
                n.ins.sync_info = mybir.SyncInfo(on_wait=[w], on_update=[])
            else:
                n.ins.sync_info.on_wait = [w]
        nc_.sync.drain()
        nc_.all_engine_barrier()
        assert self.sems is not None
        popped = nc_._tile_sem_poison_stack.pop()
        assert popped is self._sem_poison
        nc_.clear_and_free_semaphores(list(self.sems.allocated().values()))
        nc_.all_engine_barrier()

    tile.TileContext._drain_and_barrier = _drain_and_barrier


def _build_program():
    import concourse.bass as bass
    import concourse.mybir as mybir
    import concourse.tile as tile
    from concourse.vector_clock import ScopedClock

    _patch_drain(tile, mybir, ScopedClock)

    f32 = mybir.dt.float32
    bf16 = mybir.dt.bfloat16
    AF = mybir.ActivationFunctionType
    OP = mybir.AluOpType

    nc = bass.Bass("TRN2", target_bir_lowering=False, debug=False)

    # ---------------- DRAM I/O ----------------
    xT_d = nc.dram_tensor("xT", [D_MODEL, L], bf16, kind="ExternalInput")
    w_in_d = nc.dram_tensor("w_in", [D_MODEL, D_IN], bf16, kind="ExternalInput")
    w_out_d = nc.dram_tensor("w_out", [D_INNER, D_MODEL], bf16, kind="ExternalInput")
    convw_d = nc.dram_tensor("convw", [128, 10, 4], f32, kind="ExternalInput")
    convb_d = nc.dram_tensor("convb", [128, 10], f32, kind="ExternalInput")
    dtb_d = nc.dram_tensor("dtb", [16, 1], f32, kind="ExternalInput")
    nae_d = nc.dram_tensor("nae", [16, 1], f32, kind="ExternalInput")  # -exp(A_log)
    dcol_d = nc.dram_tensor("dcol", [128, 8], f32, kind="ExternalInput")  # D per pair-tile
    alow_d = nc.dram_tensor("alow", [128, 128], bf16, kind="ExternalInput")
    uinc_d = nc.dram_tensor("uinc", [128, 128], bf16, kind="ExternalInput")
    idnb_d = nc.dram_tensor("idnb", [128, 128], bf16, kind="ExternalInput")
    idnf_d = nc.dram_tensor("idnf", [128, 128], f32, kind="ExternalInput")
    ones_d = nc.dram_tensor("ones", [128, 1], bf16, kind="ExternalInput")
    onesrf_d = nc.dram_tensor("onesrf", [1, 128], f32, kind="ExternalInput")
    onesrb_d = nc.dram_tensor("onesrb", [1, 128], bf16, kind="ExternalInput")
    yT_d = nc.dram_tensor("yT", [D_MODEL, L], f32, kind="ExternalOutput")

    with tile.TileContext(nc) as tc:
        with (
            tc.tile_pool(name="const", bufs=1) as cpool,
            tc.tile_pool(name="dram", bufs=1, space="DRAM") as dpool,
            tc.tile_pool(name="mid", bufs=1) as mid,
            tc.tile_pool(name="pA", bufs=1) as pA,
        ):
            # ---------------- constants ----------------
            ALOW = cpool.tile([128, 128], bf16, tag="alow")
            nc.sync.dma_start(ALOW[:], alow_d.ap())
            UINC = cpool.tile([128, 128], bf16, tag="uinc")
            nc.sync.dma_start(UINC[:], uinc_d.ap())
            IDNB = cpool.tile([128, 128], bf16, tag="idnb")
            nc.sync.dma_start(IDNB[:], idnb_d.ap())
            IDNF = cpool.tile([128, 128], f32, tag="idnf")
            nc.sync.dma_start(IDNF[:], idnf_d.ap())
            ONEC = cpool.tile([128, 1], bf16, tag="ones")
            nc.sync.dma_start(ONEC[:], ones_d.ap())
            ONESRF = cpool.tile([1, 128], f32, tag="onesrf")
            nc.sync.dma_start(ONESRF[:], onesrf_d.ap())
            ONESRB = cpool.tile([1, 128], bf16, tag="onesrb")
            nc.sync.dma_start(ONESRB[:], onesrb_d.ap())
            CONVW = cpool.tile([128, 10, 4], f32, tag="convw")
            nc.sync.dma_start(CONVW[:], convw_d.ap())
            CONVB = cpool.tile([128, 10], f32, tag="convb")
            nc.sync.dma_start(CONVB[:], convb_d.ap())
            DTB = cpool.tile([16, 1], f32, tag="dtb")
            nc.sync.dma_start(DTB[:], dtb_d.ap())
            NAE = cpool.tile([16, 1], f32, tag="nae")
            nc.sync.dma_start(NAE[:], nae_d.ap())
            DCOL = cpool.tile([128, 8], f32, tag="dcol")
            nc.sync.dma_start(DCOL[:], dcol_d.ap())
            EPSC = cpool.tile([128, 1], f32, tag="epsc")
            nc.vector.memset(EPSC[:], EPS)

            # ---------------- persistent tensors ----------------
            dtld = mid.tile([80, L], f32, tag="dtld")           # dt 0:16, raw 32:48, logdA 64:80
            dtldT = mid.tile([128, NCH, 80], f32, tag="dtldT")  # time-major dt/logdA
            atot = mid.tile([16, 16], f32, tag="atot")          # [head, chunk]
            atotT = mid.tile([16, 16], f32, tag="atotT")        # [chunk, head]
            s_sb = [mid.tile([128, NH, HD], bf16, tag=f"s_sb{i}", name=f"s_sb{i}")
                    for i in range(2)]
            atotF = mid.tile([1, 256], f32, tag="atotF")
            wdin_all = mid.tile([128, NCH, 32], f32, tag="wdin_all")
            atb_all = mid.tile([128, NCH, 16], f32, tag="atb_all")
            sz = mid.tile([128, 8, L], bf16, tag="sz")          # raw z, then silu/g/gn

            rt_dram = dpool.tile([D_XBC, L], bf16)              # roundtrip buffer

            xTr = xT_d.ap().rearrange("(ko p) t -> p ko t", p=128)
            wir = w_in_d.ap().rearrange("(ko p) m -> p ko m", p=128)
            xTs = pA.tile([128, 4, L], bf16, tag="xTs")
            wis = pA.tile([128, 4, D_IN], bf16, tag="wis")
            for k in range(4):
                nc.sync.dma_start(xTs[:, k, :], xTr[:, k, :])
                nc.sync.dma_start(wis[:, k, :], wir[:, k, :])

            with tc.tile_pool(name="p_ysb", bufs=1) as p_ysb:
                y_sb = p_ysb.tile([128, 8, L], bf16, tag="y_sb")
                with tc.tile_pool(name="p_xbc", bufs=1) as p_xbc:
                    xbc_x = p_xbc.tile([128, 8, L], bf16, tag="xbc_x")
                    xbc_B = p_xbc.tile([128, L], bf16, tag="xbc_B")
                    xbc_C = p_xbc.tile([128, L], bf16, tag="xbc_C")

                    with (
                        tc.tile_pool(name="p_pre", bufs=2) as p_pre,
                        tc.tile_pool(name="pC", bufs=1) as pC,
                        tc.tile_pool(name="pS", bufs=2) as pS,
                        tc.tile_pool(name="pS1", bufs=1) as pS1,
                        tc.tile_pool(name="pXB", bufs=2) as pXB,
                        tc.tile_pool(name="psIn", bufs=2, space="PSUM") as psIn,
                        tc.tile_pool(name="psY", bufs=1, space="PSUM") as psY,
                        tc.tile_pool(name="psS", bufs=1, space="PSUM") as psS,
                        tc.tile_pool(name="psE", bufs=1, space="PSUM") as psE,
                        tc.tile_pool(name="psY2", bufs=1, space="PSUM") as psY2,
                    ):
                        # ---- dt block of in_proj (m = 18), all tb ----
                        for tb in range(4):
                            tsl = slice(tb * 512, (tb + 1) * 512)
                            ps = psIn.tile([128, 512], f32, tag="ps_in")
                            for k in range(4):
                                nc.tensor.matmul(
                                    ps[:16, :], wis[:, k, 18 * 128: 18 * 128 + 16],
                                    xTs[:, k, tsl], start=(k == 0), stop=(k == 3))
                            nc.scalar.copy(dtld[32:48, tsl], ps[:16, :])
                        # dt = softplus(pre) = ln(1 + exp(pre + dtb))
                        nc.scalar.activation(dtld[32:48, :], dtld[32:48, :], AF.Exp,
                                             bias=DTB[:, 0:1])
                        nc.scalar.activation(dtld[0:16, :], dtld[32:48, :], AF.Ln,
                                             bias=1.0)
                        # logdA = -exp(A_log) * dt   (f32)
                        nc.vector.tensor_scalar_mul(
                            dtld[64:80, :], dtld[0:16, :], NAE[:, 0:1])

                        # Atot per chunk = exp(chunk-sums of logdA)
                        red = psIn.tile([128, 512], f32, tag="ps_in", name="red")
                        nc.vector.tensor_reduce(
                            red[0:16, 0:16],
                            dtld[64:80, :].rearrange("p (c t) -> p c t", c=NCH),
                            op=OP.add, axis=mybir.AxisListType.X,
                        )
                        nc.scalar.activation(atot[:], red[0:16, 0:16], AF.Exp)
                        atT_ps = psIn.tile([128, 512], f32, tag="ps_in", name="atT_ps")
                        nc.tensor.transpose(
                            atT_ps[0:16, 0:16], atot[:], IDNF[0:16, 0:16])
                        nc.vector.tensor_copy(atotT[:], atT_ps[0:16, 0:16])
                        nc.sync.dma_start(
                            atotF[:].rearrange("p (c h) -> p c h", c=16), atotT[:])

                        # time-major dt/logdA per chunk via PE transpose
                        for c in range(NCH):
                            trp = psIn.tile([128, 512], f32, tag="ps_in", name="trp")
                            nc.tensor.transpose(
                                trp[:, 0:80], dtld[:, c * T:(c + 1) * T],
                                IDNF[0:80, 0:80])
                            nc.vector.tensor_copy(dtldT[:, c, :], trp[:, 0:80])

                        # ==== in_proj xBC (t-major) fused with full-L conv ====
                        for t in [8, 9] + list(range(8)):
                            m = 8 + t
                            pre = p_pre.tile([128, L + 3], bf16, tag="pre")
                            nc.vector.memset(pre[:, 0:3], 0.0)
                            for tb in range(4):
                                tsl = slice(tb * 512, (tb + 1) * 512)
                                ps = psIn.tile([128, 512], f32, tag="ps_in")
                                for k in range(4):
                                    nc.tensor.matmul(
                                        ps[:], wis[:, k, m * 128:(m + 1) * 128],
                                        xTs[:, k, tsl],
                                        start=(k == 0), stop=(k == 3))
                                nc.scalar.copy(pre[:, 3 + tb * 512: 3 + (tb + 1) * 512],
                                               ps[:])
                            acc = pC.tile([128, L], bf16, tag="conv_acc")
                            nc.vector.tensor_scalar_mul(
                                acc[:], pre[:, 0:L], CONVW[:, t, 0:1])
                            for k in (1, 2, 3):
                                nc.vector.scalar_tensor_tensor(
                                    acc[:], pre[:, k:k + L],
                                    CONVW[:, t, k:k + 1], acc[:],
                                    op0=OP.mult, op1=OP.add)
                            if t < 8:
                                dest = xbc_x[:, t, :]
                            elif t == 8:
                                dest = xbc_B[:]
                            else:
                                dest = xbc_C[:]
                            nc.scalar.activation(dest, acc[:], AF.Silu,
                                                 bias=CONVB[:, t:t + 1])
                            nc.sync.dma_start(
                                rt_dram[t * 128:(t + 1) * 128, :], dest)

                        # ---- fold D into x in-place (y eviction adds it) ----
                        for half in range(2):
                            hsl = slice(half * 1024, (half + 1) * 1024)
                            nc.vector.tensor_tensor(
                                xbc_x[:, :, hsl], xbc_x[:, :, hsl],
                                DCOL[:, :, None].to_broadcast([128, 8, 1024]),
                                op=OP.mult)

                        # ---- per-chunk decay prep (grouped Exp) ----
                        for c in range(NCH):
                            ld_bf = pS.tile([128, 16], bf16, tag="ld_bf")
                            nc.vector.tensor_copy(ld_bf[:], dtldT[:, c, 64:80])
                            wd_ps = psE.tile([128, 4, 128], f32, tag="ps_e",
                                             name="wd_ps")
                            nc.tensor.matmul(wd_ps[:, 0, 0:16], ALOW[:], ld_bf[:],
                                             start=True, stop=True)
                            nc.tensor.matmul(wd_ps[:, 0, 16:32], UINC[:], ld_bf[:],
                                             start=True, stop=True)
                            nc.scalar.activation(wdin_all[:, c, :],
                                                 wd_ps[:, 0, 0:32], AF.Exp)
                            if c > 0:
                                nc.tensor.matmul(
                                    wd_ps[:, 0, 32:48], ONESRF[:],
                                    atotF[0:1, c * 16:(c + 1) * 16],
                                    start=True, stop=True)
                                nc.vector.tensor_copy(atb_all[:, c, :],
                                                      wd_ps[:, 0, 32:48])

                        # ======================= scan =======================
                        for c in range(NCH):
                            csl = slice(c * T, (c + 1) * T)
                            wdin = wdin_all[:, c, :]

                            # scale prev state early (Pool, off critical path)
                            if c > 0:
                                s_scaled = pS.tile([128, NH, HD], bf16,
                                                   tag="s_scaled")
                                nc.gpsimd.tensor_tensor(
                                    s_scaled[:], s_sb[(c + 1) % 2][:],
                                    atb_all[:, c, :][:, :, None]
                                    .to_broadcast([128, NH, HD]),
                                    op=OP.mult)

                            xbt = pXB.tile([128, D_XBC], bf16, tag="xbt")
                            nc.sync.dma_start_transpose(xbt[:, 1024:1152],
                                                        rt_dram[1024:1152, csl])
                            nc.sync.dma_start_transpose(xbt[:, 1152:1280],
                                                        rt_dram[1152:1280, csl])
                            nc.sync.dma_start_transpose(xbt[:, 0:1024],
                                                        rt_dram[0:1024, csl])

                            # utmp16[k,h,i] = logdA[k,h] * [k<=i]  (one DVE op)
                            utmp16 = pS1.tile([128, NH, 128], bf16, tag="utmp16")
                            nc.vector.tensor_tensor(
                                utmp16[:],
                                UINC[:, None, :].to_broadcast([128, NH, 128]),
                                dtldT[:, c, 64:80][:, :, None]
                                .to_broadcast([128, NH, 128]),
                                op=OP.mult)

                            def z_block(zi):
                                zm, ztb = zi % 8, zi // 8
                                ztsl = slice(ztb * 512, (ztb + 1) * 512)
                                zps = psIn.tile([128, 512], f32, tag="ps_in")
                                for k in range(4):
                                    nc.tensor.matmul(
                                        zps[:], wis[:, k, zm * 128:(zm + 1) * 128],
                                        xTs[:, k, ztsl],
                                        start=(k == 0), stop=(k == 3))
                                nc.scalar.copy(sz[:, zm, ztsl], zps[:])

                            # Gt = B @ C^T, tril-masked (shared across heads)
                            gt_ps = psE.tile([128, 4, 128], f32, tag="ps_e",
                                             name="gt_ps")
                            nc.tensor.matmul(gt_ps[:, 0, :], xbc_B[:, csl],
                                             xbc_C[:, csl], start=True, stop=True)
                            gt = pS.tile([128, 128], bf16, tag="gt")
                            nc.vector.tensor_tensor(gt[:], gt_ps[:, 0, :],
                                                    UINC[:], op=OP.mult)

                            # segsum -> exp into m16 (z-blocks fill PE gaps)
                            m16 = pS.tile([128, NH, 128], bf16, tag="m16")
                            for hb in range(4):
                                if hb < 2:
                                    z_block(2 * c + hb)
                                e_ps = psE.tile([128, 4, 128], f32, tag="ps_e",
                                                name="e_ps")
                                nc.tensor.matmul(e_ps[:], ALOW[:],
                                                 utmp16[:, 4 * hb:4 * hb + 4, :],
                                                 start=True, stop=True)
                                nc.scalar.activation(m16[:, 4 * hb:4 * hb + 4, :],
                                                     e_ps[:], AF.Exp)
                            nc.vector.tensor_tensor(
                                m16[:], gt[:, None, :].to_broadcast([128, NH, 128]),
                                m16[:], op=OP.mult)

                            # xdt = x * dt, xdw = xdt * decay (16 heads at once)
                            xdt = pS1.tile([128, NH, HD], bf16, tag="xdt")
                            nc.vector.tensor_tensor(
                                xdt[:],
                                xbt[:, 0:1024].rearrange("p (h d) -> p h d", h=16),
                                dtldT[:, c, 0:16][:, :, None]
                                .to_broadcast([128, NH, HD]),
                                op=OP.mult)
                            xdw = pS1.tile([128, NH, HD], bf16, tag="xdw")
                            nc.gpsimd.tensor_tensor(
                                xdw[:], xdt[:],
                                wdin[:, 0:16][:, :, None]
                                .to_broadcast([128, NH, HD]),
                                op=OP.mult)

                            y_ps = psY.tile([128, 8, T], f32, tag="y_ps")
                            s_ps = psS.tile([128, NH, HD], f32, tag="s_ps")

                            if INTER_OLD:
                                for hb in range(4):
                                    if c > 0:
                                        ddiag4 = pS.tile([128, 4, 128], bf16,
                                                         tag="ddiag4")
                                        nc.gpsimd.tensor_tensor(
                                            ddiag4[:],
                                            IDNB[:, None, :]
                                            .to_broadcast([128, 4, 128]),
                                            wdin[:, 16 + 4 * hb: 20 + 4 * hb]
                                            [:, :, None].to_broadcast([128, 4, 128]),
                                            op=OP.mult)
                                        cd_ps = psE.tile([128, 4, 128], f32,
                                                         tag="ps_e", name="cd_ps")
                                        nc.tensor.matmul(cd_ps[:], xbt[:, 1152:1280],
                                                         ddiag4[:], start=True,
                                                         stop=True)
                                        cd_sb = pS.tile([128, 4, 128], bf16,
                                                        tag="cd_sb")
                                        nc.scalar.copy(cd_sb[:], cd_ps[:])
                                    for hq in range(4):
                                        h = hb * 4 + hq
                                        ph, fh = (h % 2) * 64, h // 2
                                        nc.tensor.matmul(
                                            y_ps[ph:ph + 64, fh, :],
                                            xdt[:, h, :], m16[:, h, :],
                                            start=True, stop=(c == 0))
                                        if c > 0:
                                            nc.tensor.matmul(
                                                y_ps[ph:ph + 64, fh, :],
                                                s_sb[(c + 1) % 2][:, h, :],
                                                cd_sb[:, hq, :],
                                                start=False, stop=True,
                                                skip_group_check=True)
                            else:
                                # intra-chunk: y += xdt^T @ m16 per head
                                for h in range(NH):
                                    ph, fh = (h % 2) * 64, h // 2
                                    nc.tensor.matmul(
                                        y_ps[ph:ph + 64, fh, :],
                                        xdt[:, h, :], m16[:, h, :],
                                        start=True, stop=(c == 0))
                            if c > 0 and not INTER_OLD:
                                for half in range(2):
                                    hs = slice(8 * half, 8 * half + 8)
                                    y2_ps = psY2.tile([128, 8, HD], f32,
                                                      tag="ps_y2")
                                    nc.tensor.matmul(
                                        y2_ps[:], xbc_C[:, csl],
                                        s_sb[(c + 1) % 2][:, hs, :],
                                        start=True, stop=True)
                                    y2w = pS.tile([128, 8, HD], bf16, tag="y2w")
                                    nc.vector.tensor_tensor(
                                        y2w[:], y2_ps[:],
                                        wdin[:, 16 + 8 * half: 24 + 8 * half]
                                        [:, :, None].to_broadcast([128, 8, HD]),
                                        op=OP.mult)
                                    for hq in range(8):
                                        h = 8 * half + hq
                                        ph, fh = (h % 2) * 64, h // 2
                                        nc.tensor.matmul(
                                            y_ps[ph:ph + 64, fh, :],
                                            y2w[:, hq, :], IDNB[:],
                                            start=False, stop=True,
                                            skip_group_check=True)

                            # state: S_chunk = B^T @ (x*dt*w), 8 heads per matmul
                            nc.tensor.matmul(s_ps[:, 0:8, :], xbt[:, 1024:1152],
                                             xdw[:, 0:8, :], start=True, stop=True)
                            nc.tensor.matmul(s_ps[:, 8:16, :], xbt[:, 1024:1152],
                                             xdw[:, 8:16, :], start=True,
                                             stop=True)

                            # y eviction: y = D*x + y_psum (D pre-folded into x)
                            nc.vector.tensor_tensor(
                                y_sb[:, :, csl], xbc_x[:, :, csl], y_ps[:],
                                op=OP.add)

                            # state evac: S_new = S_old * atot + S_psum
                            if c == 0:
                                nc.vector.tensor_copy(s_sb[0][:], s_ps[:])
                            else:
                                nc.vector.tensor_tensor(
                                    s_sb[c % 2][:], s_scaled[:], s_ps[:],
                                    op=OP.add)

                # ==== late phase: silu(z), gating, RMSNorm, out_proj ====
                with (
                    tc.tile_pool(name="p_late", bufs=1) as p_late,
                    tc.tile_pool(name="pG", bufs=2) as pG,
                    tc.tile_pool(name="pO", bufs=2) as pO,
                    tc.tile_pool(name="psO", bufs=2, space="PSUM") as psO,
                    tc.tile_pool(name="psN", bufs=2, space="PSUM") as psN,
                ):
                    wo = p_late.tile([128, 8, D_MODEL], bf16, tag="wo")
                    rstd_cols = p_late.tile([128, 16], f32, tag="rstd_cols")
                    rstdT = p_late.tile([16, 128], bf16, tag="rstdT")
                    rstdF = p_late.tile([1, L], bf16, tag="rstdF")
                    wor = w_out_d.ap().rearrange("(ko p) m -> p ko m", p=128)
                    nc.sync.dma_start(wo[:], wor)

                    # grouped Silu over all raw z (one table run)
                    for half in range(2):
                        hsl = slice(half * 1024, (half + 1) * 1024)
                        nc.scalar.activation(sz[:, :, hsl], sz[:, :, hsl], AF.Silu)

                    for tb in range(4):
                        tsl = slice(tb * 512, (tb + 1) * 512)
                        # gating: g = y * silu(z), in place into sz
                        nc.vector.tensor_tensor(sz[:, :, tsl], sz[:, :, tsl],
                                                y_sb[:, :, tsl], op=OP.mult)
                        # RMSNorm factors per chunk
                        for ci in range(4):
                            c = 4 * tb + ci
                            csl = slice(c * T, (c + 1) * T)
                            g2 = pG.tile([128, 8, T], bf16, tag="g2")
                            nc.scalar.activation(g2[:], sz[:, :, csl], AF.Square)
                            ssn = psN.tile([128, 128], f32, tag="ps_n")
                            for t in range(8):
                                nc.tensor.matmul(ssn[:, 0:1], g2[:, t, :], ONEC[:],
                                                 start=(t == 0), stop=(t == 7))
                            lnv = pG.tile([128, 1], f32, tag="lnv")
                            nc.scalar.activation(lnv[:], ssn[:, 0:1], AF.Ln,
                                                 bias=EPSC[:, 0:1],
                                                 scale=1.0 / D_INNER)
                            nc.scalar.activation(rstd_cols[:, c:c + 1], lnv[:],
                                                 AF.Exp, scale=-0.5)
                        # transpose + broadcast rstd over channels
                        rsn = psN.tile([128, 128], f32, tag="ps_n", name="rsn")
                        nc.tensor.transpose(rsn[0:4, 0:128],
                                            rstd_cols[:, 4 * tb:4 * tb + 4], IDNF[:])
                        nc.vector.tensor_copy(rstdT[0:4, :], rsn[0:4, 0:128])
                        nc.sync.dma_start(
                            rstdF[0:1, tsl].rearrange("p (c t) -> p c t", c=4),
                            rstdT[0:4, :])
                        rstd_bc = pG.tile([128, 512], bf16, tag="rstd_bc")
                        for ci in range(4):
                            c = 4 * tb + ci
                            rbn = psN.tile([128, 128], f32, tag="ps_n", name="rbn")
                            nc.tensor.matmul(rbn[:], ONESRB[:],
                                             rstdF[0:1, c * T:(c + 1) * T],
                                             start=True, stop=True)
                            nc.vector.tensor_copy(rstd_bc[:, ci * T:(ci + 1) * T],
                                                  rbn[:])
                        # gn = g * rstd (norm_w folded into w_out on host)
                        nc.vector.tensor_tensor(
                            sz[:, :, tsl], sz[:, :, tsl],
                            rstd_bc[:, None, :].to_broadcast([128, 8, 512]),
                            op=OP.mult)
                        # out_proj
                        for mo in range(4):
                            ps = psO.tile([128, 512], f32, tag="ps_o")
                            for k in range(8):
                                nc.tensor.matmul(
                                    ps[:], wo[:, k, mo * 128:(mo + 1) * 128],
                                    sz[:, k, tsl], start=(k == 0), stop=(k == 7))
                            yTs = pO.tile([128, 512], f32, tag="yTs")
                            nc.scalar.copy(yTs[:], ps[:])
                            nc.sync.dma_start(
                                yT_d.ap()[mo * 128:(mo + 1) * 128, tsl], yTs[:])

    _fix_waits(nc, mybir)

    return nc


def _fix_waits(nc, mybir):
    """This walrus build supports one sem-wait slot per instruction; hoist
    excess waits onto preceding NoOps on the same engine."""
    nwn = [0]
    for bb in nc.main_func.blocks:
        newl = []
        changed = False
        for inst in bb.instructions:
            si = inst.sync_info
            waits = list(si.on_wait) if (si and si.on_wait) else []
            if len(waits) > 1:
                imm = [w for w in waits if w.wait_reg is None]
                reg = [w for w in waits if w.wait_reg is not None]
                keep = (reg + imm)[:1]
                spill = [w for w in waits if w not in keep]
                assert not any(w.wait_reg is not None for w in spill), inst.name
                for w in spill:
                    nwn[0] += 1
                    nop = mybir.InstNoOp(name=f"I-wsplit-{nwn[0]}", ins=[], outs=[])
                    nop.engine = inst.engine
                    nop.sync_info = mybir.SyncInfo(on_wait=[w], on_update=[])
                    nc.register_instruction(nop)
                    newl.append(nop)
                si.on_wait = keep
                changed = True
            newl.append(inst)
        if changed:
            bb.instructions = newl
    return nc


def _get_program():
    if "nc" not in _CACHE:
        _CACHE["nc"] = _build_program()
    return _CACHE["nc"]


def _host_consts():
    if "consts" in _CACHE:
        return _CACHE["consts"]
    import ml_dtypes
    k = np.arange(128)
    alow = (k[:, None] > k[None, :]).astype(np.float32)      # [k > j]
    uinc = (k[:, None] <= k[None, :]).astype(np.float32)     # [k <= i]
    idn = np.eye(128, dtype=np.float32)
    consts = dict(
        alow=alow.astype(ml_dtypes.bfloat16),
        uinc=uinc.astype(ml_dtypes.bfloat16),
        idnb=idn.astype(ml_dtypes.bfloat16),
        idnf=idn,
        ones=np.ones((128, 1), ml_dtypes.bfloat16),
        onesrf=np.ones((1, 128), np.float32),
        onesrb=np.ones((1, 128), ml_dtypes.bfloat16),
    )
    _CACHE["consts"] = consts
    return consts


def _core_inputs(x_seq, p):
    """x_seq: (L, D_MODEL) f32 (already flipped for bw); p: dict of params."""
    import ml_dtypes
    consts = _host_consts()
    dcol = p["D"].astype(np.float32).repeat(HD).reshape(8, 128).T.copy()
    convw = np.ascontiguousarray(
        p["conv_w"].astype(np.float32).reshape(4, 10, 128).transpose(2, 1, 0)
    )
    convb = np.ascontiguousarray(p["conv_b"].astype(np.float32).reshape(10, 128).T)
    w_out = (p["norm_w"].astype(np.float32)[:, None]
             * p["out_proj"].astype(np.float32))
    return dict(
        xT=np.ascontiguousarray(x_seq.T).astype(ml_dtypes.bfloat16),
        w_in=np.ascontiguousarray(p["in_proj"]).astype(ml_dtypes.bfloat16),
        w_out=np.ascontiguousarray(w_out).astype(ml_dtypes.bfloat16),
        convw=convw,
        convb=convb,
        dtb=p["dt_bias"].astype(np.float32).reshape(16, 1),
        nae=(-np.exp(p["A_log"].astype(np.float32))).reshape(16, 1),
        dcol=dcol,
        **consts,
    )


def kernel(**inputs):
    from concourse.bass_utils import run_bass_kernel_spmd

    nc = _get_program()
    x = np.asarray(inputs["x"], np.float32)
    mask = np.asarray(inputs["padding_mask"])

    def params(pre):
        names = ["in_proj", "conv_w", "conv_b", "dt_bias", "A_log", "D", "norm_w", "out_proj"]
        return {n: np.asarray(inputs[pre + n]) for n in names}

    pf, pb = params("fw_"), params("bw_")
    in_maps = []
    for b in range(B_SZ):
        in_maps.append(_core_inputs(x[b], pf))
    for b in range(B_SZ):
        in_maps.append(_core_inputs(x[b][::-1], pb))

    res = run_bass_kernel_spmd(nc, in_maps, core_ids=list(range(8)))
    out = np.empty((B_SZ, L, D_MODEL), np.float32)
    for b in range(B_SZ):
        yf = res.results[b]["yT"].T
        yb = res.results[B_SZ + b]["yT"].T[::-1]
        out[b] = yf + yb
    out[mask] = 0.0
    return out


# revision 15
# speedup vs baseline: 1.2058x; 1.1650x over previous
"""Bidirectional Mamba2 layer on 8 NeuronCores.

Sharding: 8 cores = 4 batch elements x 2 directions (fw/bw). Each core runs
one full Mamba2 layer pass on one sequence; the host flips the bw sequences,
adds fw+bw results, and applies the padding mask.

Per-core kernel (L=2048, chunked SSD scan with T=128). Structure tuned for
TRN2 per-instruction overheads (~600ns DVE bubble, ~1us Pool floor, ~2.7us
ACT table-set switches):
  1. dt block of in_proj, softplus via the native ACT table.
  2. per xBC channel tile t: in_proj matmuls (all 4 tb) -> full-L conv taps
     (DVE) -> Silu (ACT, one table run) -> DRAM roundtrip write. in_proj of
     tile t+1 overlaps conv of tile t.
  3. chunked scan, ops batched over all 16 heads per chunk: utmp16/m16 as
     single [128,16,128] DVE ops (segsum mask folded into gt), xdt=x*dt ->
     xdw=xdt*decay chain, state matmul batched 8 heads/matmul, D*x folded
     in-place into conv output. z in_proj matmuls interleaved 2 blocks per
     chunk with raw-copy eviction (no ACT table thrash).
  4. late phase: one grouped Silu pass over z, gating in place, RMSNorm via
     Ln/Exp (one table set), out_proj per tb (norm_w folded into w_out).
"""

import numpy as np

D_MODEL = 512
D_STATE = 128
NH = 16
HD = 64
D_INNER = 1024
D_XBC = 1280
D_IN = 2320
L = 2048
T = 128
NCH = L // T
B_SZ = 4
EPS = 1e-5
INTER_OLD = False

_CACHE = {}


def _patch_drain(tile, mybir, ScopedClock):
    # workaround: this walrus build rejects >2 sem waits per instruction;
    # spread the TileContext exit-drain waits across nop instructions.
    def _drain_and_barrier(self, tick_clock, wait_clock):
        nc_ = self.nc
        probe = nc_.sync.nop()
        wait_clock.add_sem_waits(
            probe.ins, ScopedClock({None: tick_clock.global_clock})
        )
        waits = list(probe.ins.sync_info.on_wait or [])
        if probe.ins.sync_info is not None:
            probe.ins.sync_info.on_wait = waits[:1]
            rest = waits[1:]
        else:
            rest = []
        for w in rest:
            n = nc_.sync.nop()
            if n.ins.sync_info is None:
                n.ins.sync_info = mybir.SyncInfo(on_wait=[w], on_update=[])
            else:
                n.ins.sync_info.on_wait = [w]
        nc_.sync.drain()
        nc_.all_engine_barrier()
        assert self.sems is not None
        popped = nc_._tile_sem_poison_stack.pop()
        assert popped is self._sem_poison
        nc_.clear_and_free_semaphores(list(self.sems.allocated().values()))
        nc_.all_engine_barrier()

    tile.TileContext._drain_and_barrier = _drain_and_barrier


def _build_program():
    import concourse.bass as bass
    import concourse.mybir as mybir
    import concourse.tile as tile
    from concourse.vector_clock import ScopedClock

    _patch_drain(tile, mybir, ScopedClock)

    f32 = mybir.dt.float32
    bf16 = mybir.dt.bfloat16
    AF = mybir.ActivationFunctionType
    OP = mybir.AluOpType

    nc = bass.Bass("TRN2", target_bir_lowering=False, debug=False)

    # ---------------- DRAM I/O ----------------
    xT_d = nc.dram_tensor("xT", [D_MODEL, L], bf16, kind="ExternalInput")
    w_in_d = nc.dram_tensor("w_in", [D_MODEL, D_IN], bf16, kind="ExternalInput")
    w_out_d = nc.dram_tensor("w_out", [D_INNER, D_MODEL], bf16, kind="ExternalInput")
    convw_d = nc.dram_tensor("convw", [128, 10, 4], f32, kind="ExternalInput")
    convb_d = nc.dram_tensor("convb", [128, 10], f32, kind="ExternalInput")
    dtb_d = nc.dram_tensor("dtb", [16, 1], f32, kind="ExternalInput")
    nae_d = nc.dram_tensor("nae", [16, 1], f32, kind="ExternalInput")  # -exp(A_log)
    dcol_d = nc.dram_tensor("dcol", [128, 8], f32, kind="ExternalInput")  # D per pair-tile
    alow_d = nc.dram_tensor("alow", [128, 128], bf16, kind="ExternalInput")
    uinc_d = nc.dram_tensor("uinc", [128, 128], bf16, kind="ExternalInput")
    idnb_d = nc.dram_tensor("idnb", [128, 128], bf16, kind="ExternalInput")
    idnf_d = nc.dram_tensor("idnf", [128, 128], f32, kind="ExternalInput")
    ones_d = nc.dram_tensor("ones", [128, 1], bf16, kind="ExternalInput")
    onesrf_d = nc.dram_tensor("onesrf", [1, 128], f32, kind="ExternalInput")
    onesrb_d = nc.dram_tensor("onesrb", [1, 128], bf16, kind="ExternalInput")
    yT_d = nc.dram_tensor("yT", [D_MODEL, L], f32, kind="ExternalOutput")

    with tile.TileContext(nc) as tc:
        with (
            tc.tile_pool(name="const", bufs=1) as cpool,
            tc.tile_pool(name="dram", bufs=1, space="DRAM") as dpool,
            tc.tile_pool(name="mid", bufs=1) as mid,
            tc.tile_pool(name="pA", bufs=1) as pA,
        ):
            # ---------------- constants ----------------
            ALOW = cpool.tile([128, 128], bf16, tag="alow")
            nc.sync.dma_start(ALOW[:], alow_d.ap())
            UINC = cpool.tile([128, 128], bf16, tag="uinc")
            nc.sync.dma_start(UINC[:], uinc_d.ap())
            IDNB = cpool.tile([128, 128], bf16, tag="idnb")
            nc.sync.dma_start(IDNB[:], idnb_d.ap())
            IDNF = cpool.tile([128, 128], f32, tag="idnf")
            nc.sync.dma_start(IDNF[:], idnf_d.ap())
            ONEC = cpool.tile([128, 1], bf16, tag="ones")
            nc.sync.dma_start(ONEC[:], ones_d.ap())
            ONESRF = cpool.tile([1, 128], f32, tag="onesrf")
            nc.sync.dma_start(ONESRF[:], onesrf_d.ap())
            ONESRB = cpool.tile([1, 128], bf16, tag="onesrb")
            nc.sync.dma_start(ONESRB[:], onesrb_d.ap())
            CONVW = cpool.tile([128, 10, 4], f32, tag="convw")
            nc.sync.dma_start(CONVW[:], convw_d.ap())
            CONVB = cpool.tile([128, 10], f32, tag="convb")
            nc.sync.dma_start(CONVB[:], convb_d.ap())
            DTB = cpool.tile([16, 1], f32, tag="dtb")
            nc.sync.dma_start(DTB[:], dtb_d.ap())
            NAE = cpool.tile([16, 1], f32, tag="nae")
            nc.sync.dma_start(NAE[:], nae_d.ap())
            DCOL = cpool.tile([128, 8], f32, tag="dcol")
            nc.sync.dma_start(DCOL[:], dcol_d.ap())
            EPSC = cpool.tile([128, 1], f32, tag="epsc")
            nc.vector.memset(EPSC[:], EPS)

            # ---------------- persistent tensors ----------------
            dtld = mid.tile([80, L], f32, tag="dtld")           # dt 0:16, raw 32:48, logdA 64:80
            dtldT = mid.tile([128, NCH, 80], f32, tag="dtldT")  # time-major dt/logdA
            atot = mid.tile([16, 16], f32, tag="atot")          # [head, chunk]
            atotT = mid.tile([16, 16], f32, tag="atotT")        # [chunk, head]
            s_sb = [mid.tile([128, NH, HD], bf16, tag=f"s_sb{i}", name=f"s_sb{i}")
                    for i in range(2)]
            atotF = mid.tile([1, 256], f32, tag="atotF")
            wdin_all = mid.tile([128, NCH, 32], f32, tag="wdin_all")
            atb_all = mid.tile([128, NCH, 16], f32, tag="atb_all")
            sz = mid.tile([128, 8, L], bf16, tag="sz")          # raw z, then silu/g/gn

            rt_dram = dpool.tile([D_XBC, L], bf16)              # roundtrip buffer

            xTr = xT_d.ap().rearrange("(ko p) t -> p ko t", p=128)
            wir = w_in_d.ap().rearrange("(ko p) m -> p ko m", p=128)
            xTs = pA.tile([128, 4, L], bf16, tag="xTs")
            wis = pA.tile([128, 4, D_IN], bf16, tag="wis")
            for k in range(4):
                nc.sync.dma_start(xTs[:, k, :], xTr[:, k, :])
                nc.sync.dma_start(wis[:, k, :], wir[:, k, :])

            with tc.tile_pool(name="p_ysb", bufs=1) as p_ysb:
                y_sb = p_ysb.tile([128, 8, L], bf16, tag="y_sb")
                with tc.tile_pool(name="p_xbc", bufs=1) as p_xbc:
                    xbc_x = p_xbc.tile([128, 8, L], bf16, tag="xbc_x")
                    xbc_B = p_xbc.tile([128, L], bf16, tag="xbc_B")
                    xbc_C = p_xbc.tile([128, L], bf16, tag="xbc_C")

                    with (
                        tc.tile_pool(name="p_pre", bufs=2) as p_pre,
                        tc.tile_pool(name="pC", bufs=1) as pC,
                        tc.tile_pool(name="pS", bufs=2) as pS,
                        tc.tile_pool(name="pS1", bufs=1) as pS1,
                        tc.tile_pool(name="pXB", bufs=2) as pXB,
                        tc.tile_pool(name="psIn", bufs=2, space="PSUM") as psIn,
                        tc.tile_pool(name="psY", bufs=1, space="PSUM") as psY,
                        tc.tile_pool(name="psS", bufs=1, space="PSUM") as psS,
                        tc.tile_pool(name="psE", bufs=1, space="PSUM") as psE,
                        tc.tile_pool(name="psY2", bufs=1, space="PSUM") as psY2,
                    ):
                        # ---- dt block of in_proj (m = 18), all tb ----
                        for tb in range(4):
                            tsl = slice(tb * 512, (tb + 1) * 512)
                            ps = psIn.tile([128, 512], f32, tag="ps_in")
                            for k in range(4):
                                nc.tensor.matmul(
                                    ps[:16, :], wis[:, k, 18 * 128: 18 * 128 + 16],
                                    xTs[:, k, tsl], start=(k == 0), stop=(k == 3))
                            nc.scalar.copy(dtld[32:48, tsl], ps[:16, :])
                        # dt = softplus(pre) = ln(1 + exp(pre + dtb))
                        nc.scalar.activation(dtld[32:48, :], dtld[32:48, :], AF.Exp,
                                             bias=DTB[:, 0:1])
                        nc.scalar.activation(dtld[0:16, :], dtld[32:48, :], AF.Ln,
                                             bias=1.0)
                        # logdA = -exp(A_log) * dt   (f32)
                        nc.vector.tensor_scalar_mul(
                            dtld[64:80, :], dtld[0:16, :], NAE[:, 0:1])

                        # Atot per chunk = exp(chunk-sums of logdA)
                        red = psIn.tile([128, 512], f32, tag="ps_in", name="red")
                        nc.vector.tensor_reduce(
                            red[0:16, 0:16],
                            dtld[64:80, :].rearrange("p (c t) -> p c t", c=NCH),
                            op=OP.add, axis=mybir.AxisListType.X,
                        )
                        nc.scalar.activation(atot[:], red[0:16, 0:16], AF.Exp)
                        atT_ps = psIn.tile([128, 512], f32, tag="ps_in", name="atT_ps")
                        nc.tensor.transpose(
                            atT_ps[0:16, 0:16], atot[:], IDNF[0:16, 0:16])
                        nc.vector.tensor_copy(atotT[:], atT_ps[0:16, 0:16])
                        nc.sync.dma_start(
                            atotF[:].rearrange("p (c h) -> p c h", c=16), atotT[:])

                        # time-major dt/logdA per chunk via PE transpose
                        for c in range(NCH):
                            trp = psIn.tile([128, 512], f32, tag="ps_in", name="trp")
                            nc.tensor.transpose(
                                trp[:, 0:80], dtld[:, c * T:(c + 1) * T],
                                IDNF[0:80, 0:80])
                            nc.vector.tensor_copy(dtldT[:, c, :], trp[:, 0:80])

                        # ==== in_proj xBC (t-major) fused with full-L conv ====
                        for t in [8, 9] + list(range(8)):
                            m = 8 + t
                            pre = p_pre.tile([128, L + 3], bf16, tag="pre")
                            nc.vector.memset(pre[:, 0:3], 0.0)
                            for tb in range(4):
                                tsl = slice(tb * 512, (tb + 1) * 512)
                                ps = psIn.tile([128, 512], f32, tag="ps_in")
                                for k in range(4):
                                    nc.tensor.matmul(
                                        ps[:], wis[:, k, m * 128:(m + 1) * 128],
                                        xTs[:, k, tsl],
                                        start=(k == 0), stop=(k == 3))
                                nc.scalar.copy(pre[:, 3 + tb * 512: 3 + (tb + 1) * 512],
                                               ps[:])
                            acc = pC.tile([128, L], bf16, tag="conv_acc")
                            nc.vector.tensor_scalar_mul(
                                acc[:], pre[:, 0:L], CONVW[:, t, 0:1])
                            for k in (1, 2, 3):
                                nc.vector.scalar_tensor_tensor(
                                    acc[:], pre[:, k:k + L],
                                    CONVW[:, t, k:k + 1], acc[:],
                                    op0=OP.mult, op1=OP.add)
                            if t < 8:
                                dest = xbc_x[:, t, :]
                            elif t == 8:
                                dest = xbc_B[:]
                            else:
                                dest = xbc_C[:]
                            nc.scalar.activation(dest, acc[:], AF.Silu,
                                                 bias=CONVB[:, t:t + 1])
                            nc.sync.dma_start(
                                rt_dram[t * 128:(t + 1) * 128, :], dest)

                        # ---- fold D into x in-place (y eviction adds it) ----
                        for half in range(2):
                            hsl = slice(half * 1024, (half + 1) * 1024)
                            nc.vector.tensor_tensor(
                                xbc_x[:, :, hsl], xbc_x[:, :, hsl],
                                DCOL[:, :, None].to_broadcast([128, 8, 1024]),
                                op=OP.mult)

                        # ---- per-chunk decay prep (grouped Exp) ----
                        for c in range(NCH):
                            ld_bf = pS.tile([128, 16], bf16, tag="ld_bf")
                            nc.vector.tensor_copy(ld_bf[:], dtldT[:, c, 64:80])
                            wd_ps = psE.tile([128, 4, 128], f32, tag="ps_e",
                                             name="wd_ps")
                            nc.tensor.matmul(wd_ps[:, 0, 0:16], ALOW[:], ld_bf[:],
                                             start=True, stop=True)
                            nc.tensor.matmul(wd_ps[:, 0, 16:32], UINC[:], ld_bf[:],
                                             start=True, stop=True)
                            nc.scalar.activation(wdin_all[:, c, :],
                                                 wd_ps[:, 0, 0:32], AF.Exp)
                            if c > 0:
                                nc.tensor.matmul(
                                    wd_ps[:, 0, 32:48], ONESRF[:],
                                    atotF[0:1, c * 16:(c + 1) * 16],
                                    start=True, stop=True)
                                nc.vector.tensor_copy(atb_all[:, c, :],
                                                      wd_ps[:, 0, 32:48])

                        # ======================= scan =======================
                        for c in range(NCH):
                            csl = slice(c * T, (c + 1) * T)
                            wdin = wdin_all[:, c, :]

                            # scale prev state early (Pool, off critical path)
                            if c > 0:
                                s_scaled = pS.tile([128, NH, HD], bf16,
                                                   tag="s_scaled")
                                nc.gpsimd.tensor_tensor(
                                    s_scaled[:], s_sb[(c + 1) % 2][:],
                                    atb_all[:, c, :][:, :, None]
                                    .to_broadcast([128, NH, HD]),
                                    op=OP.mult)

                            xbt = pXB.tile([128, D_XBC], bf16, tag="xbt")
                            nc.sync.dma_start_transpose(xbt[:, 1024:1152],
                                                        rt_dram[1024:1152, csl])
                            nc.sync.dma_start_transpose(xbt[:, 1152:1280],
                                                        rt_dram[1152:1280, csl])
                            nc.sync.dma_start_transpose(xbt[:, 0:1024],
                                                        rt_dram[0:1024, csl])

                            # utmp16[k,h,i] = logdA[k,h] * [k<=i]  (one DVE op)
                            utmp16 = pS1.tile([128, NH, 128], bf16, tag="utmp16")
                            nc.vector.tensor_tensor(
                                utmp16[:],
                                UINC[:, None, :].to_broadcast([128, NH, 128]),
                                dtldT[:, c, 64:80][:, :, None]
                                .to_broadcast([128, NH, 128]),
                                op=OP.mult)

                            def z_block(zi):
                                zm, ztb = zi % 8, zi // 8
                                ztsl = slice(ztb * 512, (ztb + 1) * 512)
                                zps = psIn.tile([128, 512], f32, tag="ps_in")
                                for k in range(4):
                                    nc.tensor.matmul(
                                        zps[:], wis[:, k, zm * 128:(zm + 1) * 128],
                                        xTs[:, k, ztsl],
                                        start=(k == 0), stop=(k == 3))
                                nc.scalar.copy(sz[:, zm, ztsl], zps[:])

                            # Gt = B @ C^T, tril-masked (shared across heads)
                            gt_ps = psE.tile([128, 4, 128], f32, tag="ps_e",
                                             name="gt_ps")
                            nc.tensor.matmul(gt_ps[:, 0, :], xbc_B[:, csl],
                                             xbc_C[:, csl], start=True, stop=True)
                            gt = pS.tile([128, 128], bf16, tag="gt")
                            nc.vector.tensor_tensor(gt[:], gt_ps[:, 0, :],
                                                    UINC[:], op=OP.mult)

                            # segsum -> exp into m16 (z-blocks fill PE gaps)
                            m16 = pS.tile([128, NH, 128], bf16, tag="m16")
                            for hb in range(4):
                                if hb < 2:
                                    z_block(2 * c + hb)
                                e_ps = psE.tile([128, 4, 128], f32, tag="ps_e",
                                                name="e_ps")
                                nc.tensor.matmul(e_ps[:], ALOW[:],
                                                 utmp16[:, 4 * hb:4 * hb + 4, :],
                                                 start=True, stop=True)
                                nc.scalar.activation(m16[:, 4 * hb:4 * hb + 4, :],
                                                     e_ps[:], AF.Exp)
                            nc.vector.tensor_tensor(
                                m16[:], gt[:, None, :].to_broadcast([128, NH, 128]),
                                m16[:], op=OP.mult)

                            # xdt = x * dt, xdw = xdt * decay (16 heads at once)
                            xdt = pS1.tile([128, NH, HD], bf16, tag="xdt")
                            nc.vector.tensor_tensor(
                                xdt[:],
                                xbt[:, 0:1024].rearrange("p (h d) -> p h d", h=16),
                                dtldT[:, c, 0:16][:, :, None]
                                .to_broadcast([128, NH, HD]),
                                op=OP.mult)
                            xdw = pS1.tile([128, NH, HD], bf16, tag="xdw")
                            nc.gpsimd.tensor_tensor(
                                xdw[:], xdt[:],
                                wdin[:, 0:16][:, :, None]
                                .to_broadcast([128, NH, HD]),
                                op=OP.mult)

                            y_ps = psY.tile([128, 8, T], f32, tag="y_ps")
                            s_ps = psS.tile([128, NH, HD], f32, tag="s_ps")

                            if INTER_OLD:
                                for hb in range(4):
                                    if c > 0:
                                        ddiag4 = pS.tile([128, 4, 128], bf16,
                                                         tag="ddiag4")
                                        nc.gpsimd.tensor_tensor(
                                            ddiag4[:],
                                            IDNB[:, None, :]
                                            .to_broadcast([128, 4, 128]),
                                            wdin[:, 16 + 4 * hb: 20 + 4 * hb]
                                            [:, :, None].to_broadcast([128, 4, 128]),
                                            op=OP.mult)
                                        cd_ps = psE.tile([128, 4, 128], f32,
                                                         tag="ps_e", name="cd_ps")
                                        nc.tensor.matmul(cd_ps[:], xbt[:, 1152:1280],
                                                         ddiag4[:], start=True,
                                                         stop=True)
                                        cd_sb = pS.tile([128, 4, 128], bf16,
                                                        tag="cd_sb")
                                        nc.scalar.copy(cd_sb[:], cd_ps[:])
                                    for hq in range(4):
                                        h = hb * 4 + hq
                                        ph, fh = (h % 2) * 64, h // 2
                                        nc.tensor.matmul(
                                            y_ps[ph:ph + 64, fh, :],
                                            xdt[:, h, :], m16[:, h, :],
                                            start=True, stop=(c == 0))
                                        if c > 0:
                                            nc.tensor.matmul(
                                                y_ps[ph:ph + 64, fh, :],
                                                s_sb[(c + 1) % 2][:, h, :],
                                                cd_sb[:, hq, :],
                                                start=False, stop=True,
                                                skip_group_check=True)
                            elif c == 0:
                                for h in range(NH):
                                    ph, fh = (h % 2) * 64, h // 2
                                    nc.tensor.matmul(
                                        y_ps[ph:ph + 64, fh, :],
                                        xdt[:, h, :], m16[:, h, :],
                                        start=True, stop=True)
                            else:
                                # inter: y2[tok,h,hd] = C^T @ S_prev (tok-major,
                                # 8 heads/matmul), scale by incl-decay per token,
                                # transpose back into y_ps via identity matmuls.
                                # Each y_ps region opens (intra) and closes
                                # (transpose) back-to-back: the PE cannot hold
                                # many open accumulation regions per bank.
                                for half in range(2):
                                    hs = slice(8 * half, 8 * half + 8)
                                    y2_ps = psY2.tile([128, 8, HD], f32,
                                                      tag="ps_y2")
                                    nc.tensor.matmul(
                                        y2_ps[:], xbc_C[:, csl],
                                        s_sb[(c + 1) % 2][:, hs, :],
                                        start=True, stop=True)
                                    y2w = pS.tile([128, 8, HD], bf16, tag="y2w")
                                    nc.vector.tensor_tensor(
                                        y2w[:], y2_ps[:],
                                        wdin[:, 16 + 8 * half: 24 + 8 * half]
                                        [:, :, None].to_broadcast([128, 8, HD]),
                                        op=OP.mult)
                                    for hq in range(8):
                                        h = 8 * half + hq
                                        ph, fh = (h % 2) * 64, h // 2
                                        nc.tensor.matmul(
                                            y_ps[ph:ph + 64, fh, :],
                                            xdt[:, h, :], m16[:, h, :],
                                            start=True, stop=False)
                                        nc.tensor.matmul(
                                            y_ps[ph:ph + 64, fh, :],
                                            y2w[:, hq, :], IDNB[:],
                                            start=False, stop=True,
                                            skip_group_check=True)

                            # state: S_chunk = B^T @ (x*dt*w), 8 heads per matmul
                            nc.tensor.matmul(s_ps[:, 0:8, :], xbt[:, 1024:1152],
                                             xdw[:, 0:8, :], start=True, stop=True)
                            nc.tensor.matmul(s_ps[:, 8:16, :], xbt[:, 1024:1152],
                                             xdw[:, 8:16, :], start=True,
                                             stop=True)

                            # y eviction: y = D*x + y_psum (D pre-folded into x)
                            nc.vector.tensor_tensor(
                                y_sb[:, :, csl], xbc_x[:, :, csl], y_ps[:],
                                op=OP.add)

                            # state evac: S_new = S_old * atot + S_psum
                            if c == 0:
                                nc.vector.tensor_copy(s_sb[0][:], s_ps[:])
                            else:
                                nc.vector.tensor_tensor(
                                    s_sb[c % 2][:], s_scaled[:], s_ps[:],
                                    op=OP.add)

                # ==== late phase: silu(z), gating, RMSNorm, out_proj ====
                with (
                    tc.tile_pool(name="p_late", bufs=1) as p_late,
                    tc.tile_pool(name="pG", bufs=2) as pG,
                    tc.tile_pool(name="pO", bufs=2) as pO,
                    tc.tile_pool(name="psO", bufs=2, space="PSUM") as psO,
                    tc.tile_pool(name="psN", bufs=2, space="PSUM") as psN,
                ):
                    wo = p_late.tile([128, 8, D_MODEL], bf16, tag="wo")
                    rstd_cols = p_late.tile([128, 16], f32, tag="rstd_cols")
                    rstdT = p_late.tile([16, 128], bf16, tag="rstdT")
                    rstdF = p_late.tile([1, L], bf16, tag="rstdF")
                    wor = w_out_d.ap().rearrange("(ko p) m -> p ko m", p=128)
                    nc.sync.dma_start(wo[:], wor)

                    # grouped Silu over all raw z (one table run)
                    for half in range(2):
                        hsl = slice(half * 1024, (half + 1) * 1024)
                        nc.scalar.activation(sz[:, :, hsl], sz[:, :, hsl], AF.Silu)

                    for tb in range(4):
                        tsl = slice(tb * 512, (tb + 1) * 512)
                        # gating: g = y * silu(z), in place into sz
                        nc.vector.tensor_tensor(sz[:, :, tsl], sz[:, :, tsl],
                                                y_sb[:, :, tsl], op=OP.mult)
                        # RMSNorm factors per chunk
                        for ci in range(4):
                            c = 4 * tb + ci
                            csl = slice(c * T, (c + 1) * T)
                            g2 = pG.tile([128, 8, T], bf16, tag="g2")
                            nc.scalar.activation(g2[:], sz[:, :, csl], AF.Square)
                            ssn = psN.tile([128, 128], f32, tag="ps_n")
                            for t in range(8):
                                nc.tensor.matmul(ssn[:, 0:1], g2[:, t, :], ONEC[:],
                                                 start=(t == 0), stop=(t == 7))
                            lnv = pG.tile([128, 1], f32, tag="lnv")
                            nc.scalar.activation(lnv[:], ssn[:, 0:1], AF.Ln,
                                                 bias=EPSC[:, 0:1],
                                                 scale=1.0 / D_INNER)
                            nc.scalar.activation(rstd_cols[:, c:c + 1], lnv[:],
                                                 AF.Exp, scale=-0.5)
                        # transpose + broadcast rstd over channels
                        rsn = psN.tile([128, 128], f32, tag="ps_n", name="rsn")
                        nc.tensor.transpose(rsn[0:4, 0:128],
                                            rstd_cols[:, 4 * tb:4 * tb + 4], IDNF[:])
                        nc.vector.tensor_copy(rstdT[0:4, :], rsn[0:4, 0:128])
                        nc.sync.dma_start(
                            rstdF[0:1, tsl].rearrange("p (c t) -> p c t", c=4),
                            rstdT[0:4, :])
                        rstd_bc = pG.tile([128, 512], bf16, tag="rstd_bc")
                        for ci in range(4):
                            c = 4 * tb + ci
                            rbn = psN.tile([128, 128], f32, tag="ps_n", name="rbn")
                            nc.tensor.matmul(rbn[:], ONESRB[:],
                                             rstdF[0:1, c * T:(c + 1) * T],
                                             start=True, stop=True)
                            nc.vector.tensor_copy(rstd_bc[:, ci * T:(ci + 1) * T],
                                                  rbn[:])
                        # gn = g * rstd (norm_w folded into w_out on host)
                        nc.vector.tensor_tensor(
                            sz[:, :, tsl], sz[:, :, tsl],
                            rstd_bc[:, None, :].to_broadcast([128, 8, 512]),
                            op=OP.mult)
                        # out_proj
                        for mo in range(4):
                            ps = psO.tile([128, 512], f32, tag="ps_o")
                            for k in range(8):
                                nc.tensor.matmul(
                                    ps[:], wo[:, k, mo * 128:(mo + 1) * 128],
                                    sz[:, k, tsl], start=(k == 0), stop=(k == 7))
                            yTs = pO.tile([128, 512], f32, tag="yTs")
                            nc.scalar.copy(yTs[:], ps[:])
                            nc.sync.dma_start(
                                yT_d.ap()[mo * 128:(mo + 1) * 128, tsl], yTs[:])

    _fix_waits(nc, mybir)

    return nc


def _fix_waits(nc, mybir):
    """This walrus build supports one sem-wait slot per instruction; hoist
    excess waits onto preceding NoOps on the same engine."""
    nwn = [0]
    for bb in nc.main_func.blocks:
        newl = []
        changed = False
        for inst in bb.instructions:
            si = inst.sync_info
            waits = list(si.on_wait) if (si and si.on_wait) else []
            if len(waits) > 1:
                imm = [w for w in waits if w.wait_reg is None]
                reg = [w for w in waits if w.wait_reg is not None]
                keep = (reg + imm)[:1]
                spill = [w for w in waits if w not in keep]
                assert not any(w.wait_reg is not None for w in spill), inst.name
                for w in spill:
                    nwn[0] += 1
                    nop = mybir.InstNoOp(name=f"I-wsplit-{nwn[0]}", ins=[], outs=[])
                    nop.engine = inst.engine
                    nop.sync_info = mybir.SyncInfo(on_wait=[w], on_update=[])
                    nc.register_instruction(nop)
                    newl.append(nop)
                si.on_wait = keep
                changed = True
            newl.append(inst)
        if changed:
            bb.instructions = newl
    return nc


def _get_program():
    if "nc" not in _CACHE:
        _CACHE["nc"] = _build_program()
    return _CACHE["nc"]


def _host_consts():
    if "consts" in _CACHE:
        return _CACHE["consts"]
    import ml_dtypes
    k = np.arange(128)
    alow = (k[:, None] > k[None, :]).astype(np.float32)      # [k > j]
    uinc = (k[:, None] <= k[None, :]).astype(np.float32)     # [k <= i]
    idn = np.eye(128, dtype=np.float32)
    consts = dict(
        alow=alow.astype(ml_dtypes.bfloat16),
        uinc=uinc.astype(ml_dtypes.bfloat16),
        idnb=idn.astype(ml_dtypes.bfloat16),
        idnf=idn,
        ones=np.ones((128, 1), ml_dtypes.bfloat16),
        onesrf=np.ones((1, 128), np.float32),
        onesrb=np.ones((1, 128), ml_dtypes.bfloat16),
    )
    _CACHE["consts"] = consts
    return consts


def _core_inputs(x_seq, p):
    """x_seq: (L, D_MODEL) f32 (already flipped for bw); p: dict of params."""
    import ml_dtypes
    consts = _host_consts()
    dcol = p["D"].astype(np.float32).repeat(HD).reshape(8, 128).T.copy()
    convw = np.ascontiguousarray(
        p["conv_w"].astype(np.float32).reshape(4, 10, 128).transpose(2, 1, 0)
    )
    convb = np.ascontiguousarray(p["conv_b"].astype(np.float32).reshape(10, 128).T)
    w_out = (p["norm_w"].astype(np.float32)[:, None]
             * p["out_proj"].astype(np.float32))
    return dict(
        xT=np.ascontiguousarray(x_seq.T).astype(ml_dtypes.bfloat16),
        w_in=np.ascontiguousarray(p["in_proj"]).astype(ml_dtypes.bfloat16),
        w_out=np.ascontiguousarray(w_out).astype(ml_dtypes.bfloat16),
        convw=convw,
        convb=convb,
        dtb=p["dt_bias"].astype(np.float32).reshape(16, 1),
        nae=(-np.exp(p["A_log"].astype(np.float32))).reshape(16, 1),
        dcol=dcol,
        **consts,
    )


def kernel(**inputs):
    from concourse.bass_utils import run_bass_kernel_spmd

    nc = _get_program()
    x = np.asarray(inputs["x"], np.float32)
    mask = np.asarray(inputs["padding_mask"])

    def params(pre):
        names = ["in_proj", "conv_w", "conv_b", "dt_bias", "A_log", "D", "norm_w", "out_proj"]
        return {n: np.asarray(inputs[pre + n]) for n in names}

    pf, pb = params("fw_"), params("bw_")
    in_maps = []
    for b in range(B_SZ):
        in_maps.append(_core_inputs(x[b], pf))
    for b in range(B_SZ):
        in_maps.append(_core_inputs(x[b][::-1], pb))

    res = run_bass_kernel_spmd(nc, in_maps, core_ids=list(range(8)))
    out = np.empty((B_SZ, L, D_MODEL), np.float32)
    for b in range(B_SZ):
        yf = res.results[b]["yT"].T
        yb = res.results[B_SZ + b]["yT"].T[::-1]
        out[b] = yf + yb
    out[mask] = 0.0
    return out
